# revision 45
# baseline (speedup 1.0000x reference)
"""Trainium2 Bass kernel for nn_Attention (B=4, T=2048, C=1024, H=16, D=64, fp32).

Sharding: tensor-parallel over heads — 2 heads per core x 8 cores.
Each core computes:
  1. qkv projection for its 384 rows of Wqkv (x is pre-transposed + bf16-cast
     on host)
  2. causal attention for its 2 heads x 4 batches (S^T = K @ Q^T formulation,
     unnormalized softmax with the denominator computed via a ones-column in V)
  3. partial output projection (its 128 columns of the o-feature contraction)
Host sums the 8 partial outputs and adds bout.

All matmul operands are bf16 (PSUM accumulation stays fp32): measured on HW,
fp32r streams ~5x slower than bf16. The S^T matmul pads K per head to a full
128-partition contraction (other head's rows zeroed) because a 64-contraction
matmul streams at half rate on the PE.
"""

import os
import sys

import ml_dtypes
import numpy as np

for _p in ("/opt/trn_rl_repo", "/root/.axon_site/_ro/trn_rl_repo"):
    if os.path.isdir(_p) and _p not in sys.path:
        sys.path.insert(0, _p)

import concourse.tile as tile  # noqa: E402
from concourse import bacc, mybir  # noqa: E402
from concourse.bass_utils import run_bass_kernel_spmd  # noqa: E402

B, T, C = 4, 2048, 1024
H = 16
D = C // H  # 64
NCORES = 8
HPC = H // NCORES  # heads per core = 2
BT = B * T  # 8192
KT = 128  # k-tile (S^T partition dim)
QB = 512  # q-block (S^T free dim)
NKT = T // KT  # 16 k-tiles per batch
NQB = T // QB  # 4 q-blocks per batch
SCALE = 1.0 / np.sqrt(D)
MASK_BIG = 30000.0

F32 = mybir.dt.float32
BF16 = mybir.dt.bfloat16
BF16NP = ml_dtypes.bfloat16

ALLOWED, CAUSAL, GENERAL = 0, 1, 2

# PV deferral depth: how many k-tiles of slack the exp gets before the PE
# queue reaches the PV matmul that consumes its output (A/B-tunable).
PENDING = int(os.environ.get("K_PENDING", "4"))
# out-evac engine split: every ACTEVACth copy goes to ACT (0 = all DVE)
ACTEVAC = int(os.environ.get("K_ACTEVAC", "3"))
# proj-into-attn interleave stride (0 = legacy formula) and x-chunk prefetch depth
STRIDE = int(os.environ.get("K_STRIDE", "0"))
XBUFS = int(os.environ.get("K_XBUFS", "2"))
# 1 = flush each q-block's out-projection as one burst after a few tiles of
# the next q-block (covers the norm-chain latency); 0 = inline at block end
OPDEFER = int(os.environ.get("K_OPDEFER", "0"))
# 1 = build V' via the DMA hardware transpose crossbar — EXPERIMENTAL, fails
# numerically in CoreSim (xbar layout semantics differ); 0 = PE transpose +
# DVE copy (verified)
DMAT = int(os.environ.get("K_DMAT", "0"))


def _classify_mask(mask2d):
    """Per (q-block j, k-tile kt) classification, shared across (b, h).

    mask2d: [T, T] int32, mask2d[q, k] == 0 -> masked.
    Returns (plan, genbias):
      plan[j] = list of (kt, type, aux); skipped tiles omitted.
        aux = causal offset for CAUSAL, genbias index for GENERAL.
      genbias: [n_gen, 128, 512] f32 additive bias in [k, q] orientation.
    """
    plan = [[] for _ in range(NQB)]
    gen = []
    for j in range(NQB):
        q0 = j * QB
        for kt in range(NKT):
            k0 = kt * KT
            sub = mask2d[q0 : q0 + QB, k0 : k0 + KT] != 0  # [q, k]
            if not sub.any():
                continue
            if sub.all():
                plan[j].append((kt, ALLOWED, 0))
                continue
            qi = np.arange(q0, q0 + QB)[:, None]
            ki = np.arange(k0, k0 + KT)[None, :]
            off = k0 - q0
            if off in (0, 128, 256, 384) and bool((sub == (qi >= ki)).all()):
                plan[j].append((kt, CAUSAL, off))
            else:
                bias = np.where(sub, 0.0, -MASK_BIG).astype(np.float32).T  # [k, q]
                gen.append(np.ascontiguousarray(bias))
                plan[j].append((kt, GENERAL, len(gen) - 1))
    genbias = np.stack(gen) if gen else np.zeros((1, KT, QB), np.float32)
    return plan, genbias


def _build_program(plan, n_gen, loop_n=1, phases=("proj", "attn", "out"),
                   has_bias=False):
    """Build the single-core Bass program (identical across cores).

    loop_n > 1 wraps the compute in a hardware loop (benchmarking only).
    """
    nc = bacc.Bacc("TRN2", target_bir_lowering=False, debug=False)

    xT = nc.dram_tensor("xT", [C, BT], BF16, kind="ExternalInput").ap()
    wqkvT = nc.dram_tensor("wqkvT", [C, 3 * 128], BF16, kind="ExternalInput").ap()
    bqkv_s = nc.dram_tensor("bqkv_s", [3, 128], F32, kind="ExternalInput").ap()
    woutT = nc.dram_tensor("woutT", [128, C], BF16, kind="ExternalInput").ap()
    ident = nc.dram_tensor("ident", [128, 128], BF16, kind="ExternalInput").ap()
    cmask = nc.dram_tensor("cmask", [128, 128], BF16, kind="ExternalInput").ap()
    genb = nc.dram_tensor("genb", [max(n_gen, 1), KT, QB], F32, kind="ExternalInput").ap()
    partial = nc.dram_tensor("partial", [BT, C], BF16, kind="ExternalOutput").ap()

    with tile.TileContext(nc) as tc:
        _emit(tc, plan, xT, wqkvT, bqkv_s, woutT, ident, cmask, genb,
              partial, loop_n=loop_n, phases=phases, has_bias=has_bias)
    nc.compile()
    return nc


def _emit(tc, plan, xT, wqkvT, bqkv_s, woutT, ident, cmask, genb,
          partial, loop_n=1, phases=("proj", "attn", "out"), has_bias=False):
    from contextlib import ExitStack

    nc = tc.nc
    ctx = ExitStack()
    const = ctx.enter_context(tc.tile_pool(name="const", bufs=1))
    xin = ctx.enter_context(tc.tile_pool(name="xin", bufs=XBUFS))
    qkv = ctx.enter_context(tc.tile_pool(name="qkv", bufs=2))
    vpp = ctx.enter_context(tc.tile_pool(name="vp", bufs=2))
    ptile_pool = ctx.enter_context(tc.tile_pool(name="ptile", bufs=6))
    small = ctx.enter_context(tc.tile_pool(name="small", bufs=4))
    evac = ctx.enter_context(tc.tile_pool(name="evac", bufs=3))
    gbuf = ctx.enter_context(tc.tile_pool(name="gbuf", bufs=2))
    # PSUM budget, 8 banks of [128, 512] f32:
    #   ps_pm: 2 (proj accumulators / V-transposes / out-proj, tags pm0/pm1)
    #   ps_s:  4 (paired-head S^T [128, 1024] x 2 bufs)
    #   ps_o:  2 (PV accumulator per head)
    ps_pm = ctx.enter_context(tc.tile_pool(name="ps_pm", bufs=1, space="PSUM"))
    ps_s = ctx.enter_context(tc.tile_pool(name="ps_s", bufs=2, space="PSUM"))
    ps_o = ctx.enter_context(tc.tile_pool(name="ps_o", bufs=1, space="PSUM"))

    # ---- constants ----
    w_sb = const.tile([128, C // 128, 384], BF16, tag="w_sb")
    _wq = wqkvT.rearrange("(ko p) m -> p ko m", p=128)
    for _kc in range(C // 128):
        nc.gpsimd.dma_start(w_sb[:, _kc], _wq[:, _kc])
    bias_sb = const.tile([128, 3], F32, tag="bias_sb")
    nc.gpsimd.dma_start(bias_sb[:], bqkv_s.rearrange("m p -> p m"))
    wout_sb = const.tile([128, C], BF16, tag="wout_sb")
    ident_sb = const.tile([128, 128], BF16, tag="ident_sb")
    cmask_sb = const.tile([128, 128], BF16, tag="cmask_sb")

    def load_big_consts():
        nc.gpsimd.dma_start(ident_sb[:], ident)
        nc.gpsimd.dma_start(cmask_sb[:], cmask)
        nc.gpsimd.dma_start(wout_sb[:], woutT)

    nkc = C // 128
    TB = 512  # proj token-chunk; all 8 x-tiles of a chunk stay resident

    state = {}  # per-batch persistent tiles

    def proj_steps(b):
        """qkv projection + V'-build for batch b; yields after each chunk."""
        qT = qkv.tile([128, T], BF16, tag="qT", name="qT")
        # Per-head K tiles padded to 128 partitions (other head's rows zero):
        # gives the S^T matmul a full-128 contraction, which streams 2x
        # faster on the PE than a 64-contraction.
        kp = [
            qkv.tile([128, T], BF16, tag=f"kp{hh}", name=f"kp{hh}")
            for hh in range(HPC)
        ]
        vT = qkv.tile([128, T], BF16, tag="vT", name="vT")
        vp_all = vpp.tile([128, NKT, HPC, D + 1], BF16, tag="vp", name="vp")
        nc.gpsimd.memset(vp_all[:], 1.0)
        nc.gpsimd.memset(kp[0][D : 2 * D, :], 0.0)
        nc.gpsimd.memset(kp[1][0:D, :], 0.0)
        state[b] = dict(qT=qT, kp=kp, vT=vT, vp=vp_all)
        for jt in range(T // TB):
            tok0 = b * T + jt * TB
            sl = slice(jt * TB, (jt + 1) * TB)
            xch = xin.tile([128, nkc, TB], BF16, tag="xch", name="xch")
            nc.sync.dma_start(
                xch[:],
                xT[:, tok0 : tok0 + TB].rearrange("(ko p) t -> p ko t", p=128),
            )
            if b == 0 and jt == 0:
                load_big_consts()
            yield
            # q -> pm0, k -> pm1, v -> pm0 (second allocation): evacuation of
            # one group overlaps the next group's matmuls in the other bank.
            for m, tag in ((0, "pm0"), (1, "pm1"), (2, "pm0")):
                pm = ps_pm.tile([128, TB], F32, tag=tag, name="pm")
                for kc in range(nkc):
                    nc.tensor.matmul(
                        pm[:],
                        w_sb[:, kc, m * 128 : (m + 1) * 128],
                        xch[:, kc],
                        start=(kc == 0),
                        stop=(kc == nkc - 1),
                    )
                    if kc % 4 == 3:
                        yield
                if m == 0:  # q
                    if has_bias:
                        nc.vector.tensor_scalar_add(
                            qT[:, sl], pm[:], bias_sb[:, m : m + 1]
                        )
                    else:
                        nc.vector.tensor_copy(qT[:, sl], pm[:])
                elif m == 1:  # k -> two zero-padded per-head tiles
                    for hh in range(HPC):
                        p0 = hh * D
                        if has_bias:
                            nc.vector.tensor_scalar_add(
                                kp[hh][p0 : p0 + D, sl],
                                pm[p0 : p0 + D, :],
                                bias_sb[p0 : p0 + D, m : m + 1],
                            )
                        else:
                            nc.vector.tensor_copy(
                                kp[hh][p0 : p0 + D, sl], pm[p0 : p0 + D, :]
                            )
                else:  # v (on ACT: Pool cannot touch PSUM; DVE is loaded)
                    if has_bias:
                        nc.vector.tensor_scalar_add(
                            vT[:, sl], pm[:], bias_sb[:, m : m + 1]
                        )
                    else:
                        nc.scalar.copy(vT[:, sl], pm[:])
            # V' transposes for the 128-token k-tiles of this chunk
            kt0 = jt * (TB // KT)
            if DMAT:
                # DMA xbar transpose: off the PE (proj phase is PE-bound)
                for ki in range(TB // KT):
                    kt = kt0 + ki
                    # HW xbar is a plain transpose ONLY into a contiguous
                    # dst (probed on silicon: strided dsts are corrupted),
                    # so stage contiguously and let Pool scatter into vp
                    st = gbuf.tile([128, 128], BF16, tag="vstage", name="vst")
                    nc.sync.dma_start_transpose(
                        st[:], vT[:, kt * KT : (kt + 1) * KT]
                    )
                    nc.gpsimd.tensor_copy(
                        vp_all[:, kt, :, 0:D],
                        st[:].rearrange("p (h d) -> p h d", h=HPC),
                    )
                    yield
            else:
                pst = ps_pm.tile([128, TB // KT, 128], BF16, tag="pm1", name="pst")
                for ki in range(TB // KT):
                    kt = kt0 + ki
                    nc.tensor.transpose(
                        pst[:, ki, :], vT[:, kt * KT : (kt + 1) * KT], ident_sb[:]
                    )
                    yield
                nc.vector.tensor_copy(
                    vp_all[:, kt0 : kt0 + TB // KT, :, 0:D],
                    pst[:].rearrange("p f (h d) -> p f h d", h=HPC),
                )

    def outproj_steps(b, j, oT):
        """out-projection for q-block j of batch b; yields between matmuls."""
        for tp in range(2 * j, 2 * j + 2):  # pairs of 128-token tiles
            ev = evac.tile([128, 2, C], BF16, tag="ev", name="ev")
            for ti in range(2):
                tt = 2 * tp + ti
                for n in range(C // QB):
                    po = ps_pm.tile([128, QB], F32, tag=f"pm{n % 2}", name="po")
                    nc.tensor.matmul(
                        po[:],
                        oT[:, tt * 128 : (tt + 1) * 128],
                        wout_sb[:, n * QB : (n + 1) * QB],
                        start=True,
                        stop=True,
                    )
                    if ACTEVAC and (2 * tt + n) % ACTEVAC == ACTEVAC - 1:
                        nc.scalar.copy(ev[:, ti, n * QB : (n + 1) * QB], po[:])
                    else:
                        nc.vector.tensor_copy(
                            ev[:, ti, n * QB : (n + 1) * QB], po[:]
                        )
                yield
            r0 = b * T + tp * 256
            nc.sync.dma_start(
                partial[r0 : r0 + 256, :].rearrange("(ti p) c -> p ti c", p=128),
                ev[:],
            )

    def attn_steps(b):
        """attention + out-projection for batch b; yields after each k-tile."""
        st = state[b]
        qT, kp, vp_all = st["qT"], st["kp"], st["vp"]
        oT = qkv.tile([128, T], BF16, tag="oT", name="oT")
        st["oT"] = oT
        held_op = None  # deferred out-projection of the previous q-block
        for j in range(NQB):
            tiles = plan[j]
            if not tiles:
                continue
            o_ps = [
                ps_o.tile([128, QB], F32, tag=f"o{hh}", name=f"o_ps{hh}")
                for hh in range(HPC)
            ]
            def emit_pv(item):
                kt_, off_, pt_, first_, last_ = item
                for hh in range(HPC):
                    nc.tensor.matmul(
                        o_ps[hh][0 : D + 1, off_:QB],
                        vp_all[:, kt_, hh, :],
                        pt_[:, hh, off_:QB],
                        start=first_,
                        stop=last_,
                    )

            pending = []
            for idx, (kt, typ, aux) in enumerate(tiles):
                first, last = idx == 0, idx == len(tiles) - 1
                # off = width of the fully-masked q-prefix of this tile
                # (cols [0, off) have every k masked -> never computed).
                off = aux if typ == CAUSAL else 0
                sp = ps_s.tile([128, 2, QB], F32, tag="sp", name="sp")
                for hh in range(HPC):
                    nc.tensor.matmul(
                        sp[:, hh, off:QB],
                        kp[hh][:, kt * KT : (kt + 1) * KT],
                        qT[:, j * QB + off : (j + 1) * QB],
                        start=True,
                        stop=True,
                    )
                pt = ptile_pool.tile([128, 2, QB], BF16, tag="pt", name="pt")
                if typ == GENERAL:
                    gb = gbuf.tile([128, QB], F32, tag="gb", name="gb")
                    nc.sync.dma_start(gb[:], genb[aux])
                    sp2 = ps_s.tile([128, 2, QB], F32, tag="sp", name="sp2")
                    for hh in range(HPC):
                        nc.vector.tensor_add(sp2[:, hh, :], sp[:, hh, :], gb[:])
                    nc.scalar.activation(
                        pt[:], sp2[:], mybir.ActivationFunctionType.Exp, scale=SCALE
                    )
                else:
                    if off:
                        nc.gpsimd.memset(pt[:, :, 0:off], 0.0)
                    nc.scalar.activation(
                        pt[:, :, off:QB],
                        sp[:, :, off:QB],
                        mybir.ActivationFunctionType.Exp,
                        scale=SCALE,
                    )
                    if typ == CAUSAL:
                        # zero the still-masked triangle inside the window
                        nc.vector.tensor_mul(
                            pt[:, :, off : off + 128],
                            pt[:, :, off : off + 128],
                            cmask_sb[:, None, :].to_broadcast((128, HPC, 128)),
                        )
                if len(pending) >= PENDING:
                    emit_pv(pending.pop(0))
                pending.append((kt, off, pt, first, last))
                yield
            for item in pending:
                emit_pv(item)
            r1s, r64s = [], []
            for hh in range(HPC):
                r1 = small.tile([1, QB], F32, tag=f"r1{hh}", name="r1")
                nc.vector.reciprocal(r1[:], o_ps[hh][D : D + 1, :])
                r1s.append(r1)
            for hh in range(HPC):
                r64 = small.tile([D, QB], F32, tag=f"r64{hh}", name="r64")
                nc.gpsimd.partition_broadcast(r64[:], r1s[hh][:])
                r64s.append(r64)
            for hh in range(HPC):
                nc.vector.tensor_mul(
                    oT[hh * D : (hh + 1) * D, j * QB : (j + 1) * QB],
                    o_ps[hh][0:D, :],
                    r64s[hh][:],
                )
            if held_op is not None:  # previous block had <3 tiles
                for _ in held_op:
                    pass
                held_op = None
            if "out" not in phases:
                continue
            # out-projection for the token rows finalized by this q-block
            if OPDEFER:
                held_op = outproj_steps(b, j, oT)
            else:
                for _ in outproj_steps(b, j, oT):
                    yield
        if held_op is not None:
            for _ in held_op:
                pass

    # ---- software pipeline: proj(b+1) interleaved into attn(b) ----
    loop_cm = tc.For_i(0, loop_n, 1) if loop_n > 1 else None
    if loop_cm is not None:
        loop_cm.__enter__()
    for _ in proj_steps(0):
        pass
    for b in range(B):
        if "attn" not in phases:
            # phase-bisection mode: consume proj outputs with tiny copies
            st = state[b]
            dbg = evac.tile([128, QB], F32, tag="dbg", name="dbg")
            nc.vector.tensor_copy(dbg[:, 0:4], st["qT"][:, 0:4])
            nc.vector.tensor_copy(dbg[:, 4:8], st["kp"][0][:, 0:4])
            nc.vector.tensor_copy(dbg[:, 8:12], st["vp"][:, 0, :, 0:2])
            nc.sync.dma_start(partial[b : b + 1, 0:128], dbg[0:1, 0:128].bitcast(BF16)[:, 0:128])
            if b + 1 < B:
                for _ in proj_steps(b + 1):
                    pass
            continue
        attn = attn_steps(b)
        nproj = (T // TB) * 8 if b + 1 < B else 0
        proj = proj_steps(b + 1) if nproj else None
        n_attn = sum(len(plan[j]) for j in range(NQB)) + 16
        stride = STRIDE or max(1, round(n_attn / (nproj + 1))) if proj else 10 ** 9
        i = 0
        for _ in attn:
            i += 1
            if proj is not None and i % stride == 0:
                next(proj, None)
        if proj is not None:
            for _ in proj:
                pass
        if "out" not in phases:
            st = state[b]
            dbg = evac.tile([128, QB], BF16, tag="dbg2", name="dbg2")
            nc.vector.tensor_copy(dbg[:, 0:4], st["oT"][:, 0:4])
            nc.sync.dma_start(partial[b : b + 1, 0:128], dbg[0:1, 0:128])

    if loop_cm is not None:
        loop_cm.__exit__(None, None, None)
    ctx.close()


def _prep_inputs(x, mask, Wqkv, bqkv, Wout):
    x = np.asarray(x, np.float32)
    Wqkv = np.asarray(Wqkv, np.float32)
    bqkv = np.asarray(bqkv, np.float32)
    Wout = np.asarray(Wout, np.float32)
    mask2d = np.asarray(mask).reshape(T, T)

    plan, genbias = _classify_mask(mask2d)
    has_bias = bool(np.any(bqkv != 0.0))

    xT = np.ascontiguousarray(x.reshape(BT, C).T.astype(BF16NP))
    ident = np.eye(128, dtype=BF16NP)
    cmask = np.triu(np.ones((128, 128), np.float32)).astype(BF16NP)

    in_maps = []
    for core in range(NCORES):
        h0 = core * HPC * D  # first q-row of this core's heads
        wq = Wqkv[h0 : h0 + 128, :]
        wk = Wqkv[C + h0 : C + h0 + 128, :]
        wv = Wqkv[2 * C + h0 : 2 * C + h0 + 128, :]
        bv = bqkv[2 * C + h0 : 2 * C + h0 + 128]
        wqkvT = np.ascontiguousarray(np.concatenate([wq, wk, wv], 0).T.astype(BF16NP))
        bq = np.stack([bqkv[h0 : h0 + 128], bqkv[C + h0 : C + h0 + 128], bv])
        woutT = np.ascontiguousarray(Wout[:, h0 : h0 + 128].T.astype(BF16NP))
        in_maps.append({
            "xT": xT,
            "wqkvT": wqkvT,
            "bqkv_s": np.ascontiguousarray(bq.astype(np.float32)),
            "woutT": woutT,
            "ident": ident,
            "cmask": cmask,
            "genb": genbias,
        })
    return plan, genbias, in_maps, has_bias


def run(x, mask, Wqkv, bqkv, Wout, bout, trace=False, trace_kwargs=None):
    plan, genbias, in_maps, has_bias = _prep_inputs(x, mask, Wqkv, bqkv, Wout)
    nc = _build_program(plan, genbias.shape[0], has_bias=has_bias)
    res = run_bass_kernel_spmd(
        nc,
        in_maps,
        core_ids=list(range(NCORES)),
        trace=trace,
        **(trace_kwargs or {}),
    )
    acc = np.zeros((BT, C), np.float64)
    for core in range(NCORES):
        acc += res.results[core]["partial"].astype(np.float64)
    out = (acc + np.asarray(bout, np.float64)).astype(np.float32)
    return out.reshape(B, T, C), res


def kernel(x, mask, Wqkv, bqkv, Wout, bout):
    out, _ = run(x, mask, Wqkv, bqkv, Wout, bout, trace=False)
    return out
